# revision 36
# baseline (speedup 1.0000x reference)
"""Trainium2 Bass kernel for batched attention (B=8, Lq=Lk=2048, D=Dv=128).

Sharding: pure data parallel — batch element b runs on NeuronCore b.
Per-core algorithm (all matmuls bf16 with fp32 PSUM accumulation):

  qT = Wq^T @ xq^T        [d, Lq]   (PE transposes raw tiles via identity matmul)
  kT = Wk^T @ xk^T        [d, Lk]
  v  = xv @ Wv            [Lk, dv]  (natural layout, tile-major)
  for each k-tile j (16 tiles of 128):
      sT_j = kT_j^T @ qT  [128k, Lq]   (scores, TRANSPOSED: k on partitions)
      aT_j = exp(sT_j * scale + mask_bias)   (ACT, direct psum->sbuf, bf16)
      S   += aT_j                       (DVE, running column-sum helper)
      oT  += v_j^T @ aT_j [dv, Lq]      (PSUM accumulate across j)
  denT[:, t] = S_t^T @ ones             (per-q softmax denominators, [q,1] layout)
  out_t = (oT_t)^T * recip(denT_t)      (PE transpose + DVE per-partition scale)

The transposed-scores layout means the probabilities come out of the exp
already partitioned by k — exactly what the output matmul needs as rhs —
so no per-tile attention transposes are required.
"""

import sys

sys.path.insert(0, "/opt/trn_rl_repo")

import numpy as np

import concourse.bass as bass
import concourse.mybir as mybir
import concourse.tile as tile
from concourse import bacc
from concourse.bass_utils import run_bass_kernel_spmd
from concourse.masks import make_identity

P = 128
L = 2048
D = 128
T = L // P  # 16 tiles
F32 = mybir.dt.float32
I32 = mybir.dt.int32
BF16 = mybir.dt.bfloat16
SCALE = 1.0 / float(np.sqrt(128.0))
N_CORES = 8

ADD = mybir.AluOpType.add
MULT = mybir.AluOpType.mult
EXP = mybir.ActivationFunctionType.Exp


def build():
    nc = bacc.Bacc("TRN2", target_bir_lowering=False, debug=False)

    q_ext = nc.declare_dram_parameter("query", [L, D], F32, isOutput=False)
    k_ext = nc.declare_dram_parameter("key", [L, D], F32, isOutput=False)
    v_ext = nc.declare_dram_parameter("value", [L, D], F32, isOutput=False)
    wq_ext = nc.declare_dram_parameter("Wq", [D, D], F32, isOutput=False)
    wk_ext = nc.declare_dram_parameter("Wk", [D, D], F32, isOutput=False)
    wv_ext = nc.declare_dram_parameter("Wv", [D, D], F32, isOutput=False)
    m_ext = nc.declare_dram_parameter("mask", [1, L], I32, isOutput=False)
    out_ext = nc.declare_dram_parameter("out", [L, D], BF16, isOutput=True)

    with tile.TileContext(nc) as tc:
        with (
            tc.tile_pool(name="const", bufs=1) as const,
            tc.tile_pool(name="big", bufs=1) as big,
            tc.tile_pool(name="stage", bufs=2) as stage,
            tc.tile_pool(name="xstage", bufs=3) as xstage,
            tc.tile_pool(name="att", bufs=16) as att,
            tc.tile_pool(name="outp", bufs=4) as outp,
            # [128,1024]f32 slots x3 = 6 banks; all non-oT psum shares these
            tc.tile_pool(name="ps", bufs=3, space="PSUM") as ps,
            # one [128,1024]f32 slot = 2 banks; per-half output accumulator
            tc.tile_pool(name="ps_o", bufs=1, space="PSUM") as ps_o,
        ):
            # ---- PE warm-up: flip the HAM clock gate to 8/8 while DMAs run ----
            warm = const.tile([P, P], BF16, tag="warm")
            nc.gpsimd.memset(warm[:], 0.125)
            wps = ps.tile([P, 512], F32, tag="ps", name="warmps")
            for _ in range(36):
                nc.tensor.matmul(wps[:, :P], warm[:], warm[:], start=True, stop=True)
            # preload the exp table set (~2.7us) while DMAs are in flight
            dummy_exp = const.tile([P, 1], F32, tag="dummy")
            nc.scalar.activation(dummy_exp[:], warm[:, :1], EXP)

            # ---- gpsimd first: constants, weights, mask (tiny DMAs) ----
            ident_f = stage.tile([P, P], F32, tag="identf")
            make_identity(nc, ident_f[:])
            ident = const.tile([P, P], BF16, tag="ident")
            nc.vector.tensor_copy(out=ident[:], in_=ident_f[:])

            ones_col = const.tile([P, 1], BF16, tag="ones")
            nc.gpsimd.memset(ones_col[:], 1.0)

            w_bf = {}
            for name, ext in (("Wk", wk_ext), ("Wq", wq_ext), ("Wv", wv_ext)):
                wf = stage.tile([P, D], F32, tag="wstage")
                nc.gpsimd.dma_start(wf[:], ext[:])
                wb = const.tile([P, D], BF16, tag=f"w_{name}")
                nc.vector.tensor_copy(out=wb[:], in_=wf[:])
                w_bf[name] = wb

            # mask bias: element k=(16*p+t) -> [p, t]; bias = (m-1)*1e4
            mask_i = const.tile([P, T], I32, tag="maski")
            nc.gpsimd.dma_start(
                mask_i[:], m_ext[:].rearrange("o (p t) -> p (o t)", p=P)
            )
            mask_bias = const.tile([P, T], F32, tag="maskb")
            nc.vector.tensor_scalar(
                mask_bias[:], mask_i[:], 10000.0, -10000.0, MULT, ADD
            )

            # ---- big input DMAs: priority order per queue ----
            # sync (HWDGE):   k.h0, q.h0, v.h0
            # gpsimd (SWDGE): q.h1, v.h1, k.h1   (k.h1 not needed until j=8)
            H = T // 2
            xf = {}
            srcs = {}
            for name, ext in (("k", k_ext), ("q", q_ext), ("v", v_ext)):
                xf[name] = [
                    xstage.tile([P, H, D], F32, tag="xf32", bufs=3, name=f"xf_{name}{h}")
                    for h in range(2)
                ]
                srcs[name] = ext[:].rearrange("(p t) d -> p t d", p=P)
            for name in ("k", "q", "v"):
                nc.sync.dma_start(xf[name][0][:], srcs[name][:, :H, :])
            # k.h1 first (needed by pass0 j=8); q.h1 last (only pass 1 needs it)
            for name in ("k", "v", "q"):
                nc.gpsimd.dma_start(xf[name][1][:], srcs[name][:, H:, :])

            # ---- persistent big tensors (half-granular for fine deps) ----
            qT_h = [big.tile([P, 1024], BF16, tag=f"qT{h}", name=f"qT{h}") for h in range(2)]
            kT_h = [big.tile([P, 1024], BF16, tag=f"kT{h}", name=f"kT{h}") for h in range(2)]
            v_h = [big.tile([P, 1024], BF16, tag=f"v{h}", name=f"v{h}") for h in range(2)]
            S_h = [big.tile([P, 1024], BF16, tag=f"S{h}", name=f"S{h}") for h in range(2)]

            def transpose_half(name, h, use_act_copy):
                """xf half -> bf16 cast -> 8 PE transposes -> xT_h [d, 8, p]."""
                xb = xstage.tile(
                    [P, H, D], BF16, tag="xbf", bufs=3, name=f"xb_{name}{h}"
                )
                nc.vector.tensor_copy(
                    out=xb[:].rearrange("p a b -> p (a b)"),
                    in_=xf[name][h][:].rearrange("p a b -> p (a b)"),
                )
                xT = stage.tile(
                    [P, H, P], BF16, tag=f"xT_{name}{h}", name=f"xT_{name}{h}"
                )
                pst = ps.tile([P, 8 * P], F32, tag="ps", name=f"tp_{name}{h}")
                for c in range(8):
                    nc.tensor.matmul(
                        pst[:, c * P : (c + 1) * P],
                        xb[:, c, :],
                        ident[:],
                        start=True,
                        stop=True,
                    )
                if use_act_copy:
                    nc.scalar.copy(out=xT[:], in_=pst[:])
                else:
                    nc.vector.tensor_copy(out=xT[:], in_=pst[:])
                return xT

            def qk_proj_from(xT, wname, dst_h, name, h, use_act_copy=False):
                pst = ps.tile([P, 1024], F32, tag="ps", name=f"pj_{name}{h}")
                for c in range(2):
                    nc.tensor.matmul(
                        pst[:, 512 * c : 512 * (c + 1)],
                        w_bf[wname][:],
                        xT[:, 4 * c : 4 * c + 4, :],
                        start=True,
                        stop=True,
                    )
                if use_act_copy:
                    nc.scalar.copy(out=dst_h[:], in_=pst[:])
                else:
                    nc.vector.tensor_copy(out=dst_h[:], in_=pst[:])

            def qk_prep_half(name, wname, dst_h, h, use_act_copy=False):
                xT = transpose_half(name, h, use_act_copy)
                qk_proj_from(xT, wname, dst_h, name, h, use_act_copy)

            def v_proj_from(xT, h):
                pst = ps.tile([P, 8 * P], F32, tag="ps", name=f"pjv{h}")
                for c in range(8):
                    nc.tensor.matmul(
                        pst[:, c * P : (c + 1) * P],
                        xT[:, c, :],
                        w_bf["Wv"][:],
                        start=True,
                        stop=True,
                    )
                nc.vector.tensor_copy(out=v_h[h][:], in_=pst[:])

            def v_prep_half(h):
                v_proj_from(transpose_half("v", h, use_act_copy=False), h)

            def fillers(n):
                # keep the PE busy (and the HAM clock-gate open) across
                # dependency stalls in prep
                for _ in range(n):
                    nc.tensor.matmul(
                        wps[:, :P], warm[:], warm[:], start=True, stop=True
                    )

            # prep that must precede the loop: kT half 0, qT both halves
            qk_prep_half("k", "Wk", kT_h[0], 0)
            fillers(14)
            qk_prep_half("q", "Wq", qT_h[0], 0)

            # ---- main loop: two passes over q-halves ----
            # Per pass the oT accumulator is [128,1024] (2 banks), which
            # frees a third sT psum slot (elasticity for the exp chain),
            # and lets the h0 epilogue overlap pass 1 entirely.
            out_all = big.tile([P, T, D], BF16, tag="out_all")
            out_dst = out_ext[:].rearrange("(p t) d -> p t d", p=P)
            COPY_FN = mybir.ActivationFunctionType.Copy
            oT_hs = {}

            def emit_oT(h, j, a_list):
                for c in range(2):
                    nc.tensor.matmul(
                        oT_hs[h][:, c * 512 : (c + 1) * 512],
                        v_h[j // 8][:, (j % 8) * D : (j % 8 + 1) * D],
                        a_list[j][:, c * 512 : (c + 1) * 512],
                        start=(j == 0),
                        stop=(j == T - 1),
                    )

            def emit_epilogue_half(h):
                # denominators for this q-half, already [q,1] partition layout
                dps = ps.tile([P, 8], F32, tag="ps", name=f"dps{h}")
                for tt in range(8):
                    nc.tensor.matmul(
                        dps[:, tt : tt + 1],
                        S_h[h][:, tt * P : (tt + 1) * P],
                        ones_col[:],
                        start=True,
                        stop=True,
                    )
                denT = const.tile([P, 8], F32, tag=f"denT{h}", name=f"denT{h}")
                nc.vector.tensor_copy(out=denT[:], in_=dps[:])
                rT = const.tile([P, 8], F32, tag=f"rT{h}", name=f"rT{h}")
                nc.vector.reciprocal(rT[:], denT[:])
                oT_bf = big.tile([P, 1024], BF16, tag=f"oTb{h}", name=f"oTb{h}")
                nc.vector.tensor_copy(out=oT_bf[:], in_=oT_hs[h][:])
                for g in range(2):
                    tps = ps.tile([P, 4 * P], F32, tag="ps", name=f"tps{h}_{g}")
                    for c in range(4):
                        tt = g * 4 + c
                        nc.tensor.matmul(
                            tps[:, c * P : (c + 1) * P],
                            oT_bf[:, tt * P : (tt + 1) * P],
                            ident[:],
                            start=True,
                            stop=True,
                        )
                    for c in range(4):
                        tt = g * 4 + c
                        tg = 8 * h + tt
                        nc.vector.tensor_scalar_mul(
                            out_all[:, tg, :],
                            tps[:, c * P : (c + 1) * P],
                            rT[:, tt : tt + 1],
                        )
                    eng = nc.sync if (h + g) % 2 == 0 else nc.gpsimd
                    eng.dma_start(
                        out_dst[:, 8 * h + 4 * g : 8 * h + 4 * (g + 1), :],
                        out_all[:, 8 * h + 4 * g : 8 * h + 4 * (g + 1), :],
                    )

            OT_LAG = 5
            prep_xT = {}
            for h in range(2):
                oT_hs[h] = ps_o.tile([P, 1024], F32, tag="oT", name=f"oT{h}")
                a_list = []
                for j in range(T):
                    sps = ps.tile([P, 1024], F32, tag="ps", name=f"sT{h}_{j}")
                    for c in range(2):
                        nc.tensor.matmul(
                            sps[:, c * 512 : (c + 1) * 512],
                            kT_h[j // 8][:, (j % 8) * P : (j % 8 + 1) * P],
                            qT_h[h][:, c * 512 : (c + 1) * 512],
                            start=True,
                            stop=True,
                        )
                    a = att.tile([P, 1024], BF16, tag="aT", name=f"aT{h}_{j}")
                    nc.scalar.activation(
                        a[:],
                        sps[:],
                        EXP,
                        bias=mask_bias[:, j : j + 1],
                        scale=SCALE,
                    )
                    a_list.append(a)
                    if j == 0:
                        nc.vector.tensor_copy(out=S_h[h][:], in_=a[:])
                    else:
                        nc.vector.tensor_tensor(S_h[h][:], S_h[h][:], a[:], ADD)
                    if h == 0:
                        if j == 1:
                            prep_xT["k1"] = transpose_half("k", 1, False)
                        if j == 2:
                            qk_proj_from(prep_xT["k1"], "Wk", kT_h[1], "k", 1)
                        if j == 3:
                            prep_xT["v0"] = transpose_half("v", 0, False)
                        if j == 4:
                            v_proj_from(prep_xT["v0"], 0)
                        if j == 8:
                            prep_xT["q1"] = transpose_half("q", 1, False)
                        if j == 9:
                            qk_proj_from(prep_xT["q1"], "Wq", qT_h[1], "q", 1)
                        if j == 10:
                            prep_xT["v1"] = transpose_half("v", 1, False)
                        if j == 11:
                            v_proj_from(prep_xT["v1"], 1)
                    else:
                        if j == 1:
                            emit_epilogue_half(0)
                    if j >= OT_LAG:
                        emit_oT(h, j - OT_LAG, a_list)
                for jj in range(T - OT_LAG, T):
                    emit_oT(h, jj, a_list)
            emit_epilogue_half(1)

    nc.compile()
    return nc


_NC_CACHE = None


def _get_nc():
    global _NC_CACHE
    if _NC_CACHE is None:
        _NC_CACHE = build()
    return _NC_CACHE


def kernel(query, key, value, Wq, Wk, Wv, attention_mask):
    query = np.asarray(query, dtype=np.float32)
    key = np.asarray(key, dtype=np.float32)
    value = np.asarray(value, dtype=np.float32)
    Wq = np.asarray(Wq, dtype=np.float32)
    Wk = np.asarray(Wk, dtype=np.float32)
    Wv = np.asarray(Wv, dtype=np.float32)
    mask = np.asarray(attention_mask, dtype=np.int32).reshape(N_CORES, 1, L)

    nc = _get_nc()
    in_maps = [
        {
            "query": np.ascontiguousarray(query[b]),
            "key": np.ascontiguousarray(key[b]),
            "value": np.ascontiguousarray(value[b]),
            "Wq": Wq,
            "Wk": Wk,
            "Wv": Wv,
            "mask": np.ascontiguousarray(mask[b]),
        }
        for b in range(N_CORES)
    ]
    res = run_bass_kernel_spmd(nc, in_maps, core_ids=list(range(N_CORES)))
    out = np.stack(
        [np.asarray(res.results[b]["out"]) for b in range(N_CORES)], axis=0
    )
    return out.astype(np.float32)


if __name__ == "__main__":
    rng = np.random.default_rng(0)
    q = rng.standard_normal((N_CORES, L, D), dtype=np.float32)
    k = rng.standard_normal((N_CORES, L, D), dtype=np.float32)
    v = rng.standard_normal((N_CORES, L, D), dtype=np.float32)
    wq = rng.standard_normal((D, D), dtype=np.float32) * 0.08
    wk = rng.standard_normal((D, D), dtype=np.float32) * 0.08
    wv = rng.standard_normal((D, D), dtype=np.float32) * 0.08
    m = np.ones((N_CORES, 1, L), dtype=np.int32)
    out = kernel(
        query=q, key=k, value=v, Wq=wq, Wk=wk, Wv=wv, attention_mask=m
    )
    print(out.shape, out.dtype)
